# revision 4
# baseline (speedup 1.0000x reference)
"""Deformable (region-aware) matmul for Trainium2, data-parallel over batch.

out[b,o,h,w] = sum_r sum_c mat0[b,c,h,w] * mat1[o,c,r] * Alpha[r] * mask[r,h,w]

Shapes: B=8, C=256, H=W=64, O=256, R=8.  One batch per NeuronCore (8 cores).

Fold (region, channel-half) into a 16-tile contraction axis: k-tile
t = 2*r + half.  Activation tile X_t[k, p] = mat0[half*128+k, p] * mask[r, p]
(Vector engine, bf16 2x, broadcast APs against a host-prebroadcast mask).
Weight tile W_t[k, o] = mat1[o, half*128+k, r]*Alpha[r] (lhsT layout).  The
Tensor engine chains out[o, p] = sum_t W_t.T @ X_t in PSUM at the bf16
roofline (213 ns / 512-col matmul, ~55 us/core total).

The schedule is built around the measured fixed costs (6.5 us NEFF preamble,
1.6 us DMA first-byte, ~2 us completion receipt): the pixel axis is cut into
slabs (256,256,512*6,256,256); slab0's whole working set rides in two
fat-descriptor "prologue bundles" on the sync ring (weights for the first
output half + x + mask interleaved) so the MM stream starts ~3.5 us after
the preamble; steady slabs are single [x|mask] bundle DMAs on the scalar
ring; weights are split by output half so only 0.5 MB gates the first chain.
PE warmup matmuls cover the prologue (HAM at 2.4 GHz when the stream
starts); output is stored bf16 and the last slab is small so the final DMA
drains quickly.
"""

import numpy as np
import ml_dtypes

B, C, H, W_ = 8, 256, 64, 64
O, R = 256, 8
P = H * W_            # 4096 pixels
KT = 2 * R            # 16 k-tiles of 128
SL = [256, 256, 512, 512, 512, 512, 512, 512, 256, 256]   # slab pixel counts
assert sum(SL) == P
SOFF = [0] * len(SL)
for _i in range(1, len(SL)):
    SOFF[_i] = SOFF[_i - 1] + SL[_i - 1]
NWARM = 7             # PE warmup matmuls (cover the ~3.3 us data lead-in)
MAXPX = max(SL)
PX0 = SL[0]
NS = len(SL)

_CACHE = {}


def _build():
    import concourse.bacc as bacc
    import concourse.tile as tile
    import concourse.mybir as mybir

    bf16 = mybir.dt.bfloat16
    f32 = mybir.dt.float32

    nc = bacc.Bacc(
        "TRN2",
        target_bir_lowering=False,
        debug=False,
        enable_asserts=False,
        num_devices=8,
    )
    # Host-prepped per-core inputs (see _prep_inputs):
    #  pba[k, :]  = [ w_m0 t0-7 (8*128) | x_s0 (2*PX0) | mb_s0 r0-3 (4*PX0) ]
    #  pbb[k, :]  = [ w_m0 t8-15 (8*128) | mb_s0 r4-7 (4*PX0) ]
    #  wm[k, m*KT*128 + t*128 + o'] = mat1[m*128+o', c(t,k), r(t)] * Alpha
    #  bz[k, :]   = per-slab s>=1: [ x (2*px) | mb (8*px) ] concatenated
    #  yz[k, 2*soff + m*px + i] = out[m*128+k, soff+i]  (bf16)
    pba_d = nc.dram_tensor("pba", [128, 8 * 128 + 6 * PX0], bf16, kind="ExternalInput")
    pbb_d = nc.dram_tensor("pbb", [128, 8 * 128 + 4 * PX0], bf16, kind="ExternalInput")
    wm_d = nc.dram_tensor("wm", [128, 2 * KT * 128], bf16, kind="ExternalInput")
    BZTOT = sum(10 * px for px in SL[1:])
    bz_d = nc.dram_tensor("bz", [128, BZTOT], bf16, kind="ExternalInput")
    y_d = nc.dram_tensor("yz", [128, 2 * P], bf16, kind="ExternalOutput")

    boff = [0] * NS  # offsets into bz_d per slab (s>=1)
    for s in range(2, NS):
        boff[s] = boff[s - 1] + 10 * SL[s - 1]

    with tile.TileContext(nc) as tc:
        with (
            tc.tile_pool(name="const", bufs=1) as cpool,
            tc.tile_pool(name="bzp", bufs=4) as bzpool,
            tc.tile_pool(name="xtp", bufs=4) as xtpool,
            tc.tile_pool(name="psp", bufs=8, space="PSUM") as pspool,
            tc.tile_pool(name="yp", bufs=3) as ypool,
        ):
            # --- prologue DMAs on the sync ring, in consumption order
            pba = cpool.tile([128, 8 * 128 + 6 * PX0], bf16, tag="pba")
            pbb = cpool.tile([128, 8 * 128 + 4 * PX0], bf16, tag="pbb")
            wm = cpool.tile([128, 2 * KT * 128], bf16, tag="wm")
            nc.sync.dma_start(out=pba[:], in_=pba_d[:])
            nc.sync.dma_start(out=pbb[:], in_=pbb_d[:])
            nc.sync.dma_start(out=wm[:, KT * 128 :], in_=wm_d[:, KT * 128 :])  # m1
            nc.sync.dma_start(out=wm[:, : KT * 128], in_=wm_d[:, : KT * 128])  # m0

            # --- steady-slab bundles on the scalar ring
            bz_sb = [None] * NS
            for s in range(1, NS):
                px = SL[s]
                t_ = bzpool.tile([128, 10 * MAXPX], bf16, tag="bz")
                nc.scalar.dma_start(
                    out=t_[:, : 10 * px], in_=bz_d[:, boff[s] : boff[s] + 10 * px]
                )
                bz_sb[s] = t_

            # --- PE warmup (vector memset of one combined tile keeps it cheap)
            warm = cpool.tile([128, 640], bf16, tag="warm")
            nc.vector.memset(warm[:], 0.0)
            warm_ps = pspool.tile([128, 512], f32, tag="ps")
            for _ in range(NWARM):
                nc.tensor.matmul(
                    warm_ps[:], warm[:, :128], warm[:, 128:640], start=True, stop=True
                )

            XW = 8 * 128  # weight prefix length in pba/pbb

            def lhsT(s, m, t):
                if s == 0 and m == 0:
                    src = pba if t < 8 else pbb
                    return src[:, (t % 8) * 128 : (t % 8 + 1) * 128]
                return wm[:, m * KT * 128 + t * 128 : m * KT * 128 + (t + 1) * 128]

            for s, px in enumerate(SL):
                if s == 0:
                    x_ap = pba[:, XW : XW + 2 * px]
                    mb_lo = pba[:, XW + 2 * px : XW + 6 * px]
                    mb_hi = pbb[:, XW : XW + 4 * px]
                    gsrc = [(0, 2, mb_lo), (2, 4, mb_lo), (4, 6, mb_hi), (6, 8, mb_hi)]
                else:
                    bzt = bz_sb[s]
                    x_ap = bzt[:, : 2 * px]
                    mb = bzt[:, 2 * px : 10 * px]
                    if s == 1:
                        gsrc = [(0, 2, mb), (2, 4, mb), (4, 6, mb), (6, 8, mb)]
                    else:
                        gsrc = [(0, 4, mb), (4, 8, mb)]
                xt = xtpool.tile([128, KT * MAXPX], bf16, tag="xt")
                for lo, hi, mbsrc in gsrc:
                    nr = hi - lo
                    # mbsrc holds rows [blo..] where blo = 0 for pba/bz, 4 for pbb
                    blo = 4 if (s == 0 and mbsrc is mb_hi) else 0
                    out_ap = (
                        xt[:, 2 * lo * px : 2 * hi * px]
                        .rearrange("q (r h f) -> q r h f", r=nr, h=2)
                    )
                    in0 = (
                        x_ap.rearrange("q (h f) -> q h f", h=2)
                        .unsqueeze(1)
                        .broadcast_to([128, nr, 2, px])
                    )
                    in1 = (
                        mbsrc[:, (lo - blo) * px : (hi - blo) * px]
                        .rearrange("q (r f) -> q r f", r=nr)
                        .unsqueeze(2)
                        .broadcast_to([128, nr, 2, px])
                    )
                    nc.vector.tensor_mul(out_ap, in0, in1)
                y_sb = ypool.tile([128, 2 * MAXPX], bf16, tag="y")
                yo = 2 * SOFF[s]
                for m in range(2):
                    ps = pspool.tile([128, 512], f32, tag="ps")
                    for t in range(KT):
                        nc.tensor.matmul(
                            ps[:, :px],
                            lhsT(s, m, t),
                            xt[:, t * px : (t + 1) * px],
                            start=(t == 0),
                            stop=(t == KT - 1),
                        )
                    nc.scalar.copy(y_sb[:, m * px : (m + 1) * px], ps[:, :px])
                    if s == NS - 1:  # split the final writes so the last is tiny
                        nc.sync.dma_start(
                            out=y_d[:, yo + m * px : yo + (m + 1) * px],
                            in_=y_sb[:, m * px : (m + 1) * px],
                        )
                if s != NS - 1:
                    nc.sync.dma_start(
                        out=y_d[:, yo : yo + 2 * px], in_=y_sb[:, : 2 * px]
                    )

    nc.compile()
    return nc


def _prep_inputs(mat0, mat1, mask, Alpha, use_alpha):
    bf = ml_dtypes.bfloat16
    m1 = mat1 * np.asarray(Alpha)[None, None, :] if int(use_alpha) else mat1
    # w[k, t, o] with t = r*2 + half, c = half*128 + k
    w = np.transpose(m1.reshape(O, 2, 128, R), (2, 3, 1, 0))  # [k, r, half, o]
    w = w.reshape(128, KT, O)
    # wm[k, m, t, o'] -> [128, 2*KT*128]
    wm = np.transpose(w.reshape(128, KT, 2, 128), (0, 2, 1, 3))
    wm_h = np.ascontiguousarray(wm.reshape(128, 2 * KT * 128)).astype(bf)

    mask2 = np.asarray(mask, dtype=np.float32).reshape(R, P)
    xr = np.asarray(mat0, dtype=np.float32).reshape(B, 2, 128, P)

    def xblk(b, s):  # [128, 2*px] = x halves for slab s
        p0, px = SOFF[s], SL[s]
        return (
            np.transpose(xr[b, :, :, p0 : p0 + px], (1, 0, 2))
            .reshape(128, 2 * px)
            .astype(bf)
        )

    def mblk(s, lo, hi):  # [128, (hi-lo)*px] mask rows broadcast
        p0, px = SOFF[s], SL[s]
        return (
            np.broadcast_to(mask2[None, lo:hi, p0 : p0 + px], (128, hi - lo, px))
            .reshape(128, (hi - lo) * px)
            .astype(bf)
        )

    wq = wm_h.reshape(128, 2, KT, 128)
    pba = np.empty((B, 128, 8 * 128 + 6 * SL[0]), dtype=bf)
    pbb = np.empty((128, 8 * 128 + 4 * SL[0]), dtype=bf)
    pbb[:, : 8 * 128] = wq[:, 0, 8:, :].reshape(128, 8 * 128)
    pbb[:, 8 * 128 :] = mblk(0, 4, 8)
    mb03 = mblk(0, 0, 4)
    for b in range(B):
        pba[b, :, : 8 * 128] = wq[:, 0, :8, :].reshape(128, 8 * 128)
        pba[b, :, 8 * 128 : 8 * 128 + 2 * SL[0]] = xblk(b, 0)
        pba[b, :, 8 * 128 + 2 * SL[0] :] = mb03

    BZTOT = sum(10 * px for px in SL[1:])
    bz = np.empty((B, 128, BZTOT), dtype=bf)
    off = 0
    for s in range(1, NS):
        px = SL[s]
        mb = mblk(s, 0, 8)
        for b in range(B):
            bz[b, :, off : off + 2 * px] = xblk(b, s)
            bz[b, :, off + 2 * px : off + 10 * px] = mb
        off += 10 * px
    return pba, pbb, wm_h, bz


def _decode_y(yz):
    # yz [128, 2*P] bf16 slab-major -> out [O, P] float32
    out = np.empty((O, P), dtype=np.float32)
    y = np.asarray(yz)
    for s, px in enumerate(SL):
        p0 = SOFF[s]
        blk = y[:, 2 * p0 : 2 * (p0 + px)].reshape(128, 2, px).astype(np.float32)
        out[:128, p0 : p0 + px] = blk[:, 0, :]
        out[128:, p0 : p0 + px] = blk[:, 1, :]
    return out


def _make_in_maps(mat0, mat1, mask, Alpha, use_alpha):
    pba, pbb, wm_h, bz = _prep_inputs(mat0, mat1, mask, Alpha, use_alpha)
    return [
        {"pba": pba[b], "pbb": pbb, "wm": wm_h, "bz": bz[b]} for b in range(B)
    ]


def kernel(mat0, mat1, mask, Alpha, use_alpha, beta):
    from concourse import bass_utils

    mat0 = np.asarray(mat0, dtype=np.float32)
    mat1 = np.asarray(mat1, dtype=np.float32)
    mask = np.asarray(mask, dtype=np.float32)
    Alpha = np.asarray(Alpha, dtype=np.float32)

    if "nc" not in _CACHE:
        _CACHE["nc"] = _build()
    nc = _CACHE["nc"]

    in_maps = _make_in_maps(mat0, mat1, mask, Alpha, use_alpha)
    res = bass_utils.run_bass_kernel_spmd(nc, in_maps, core_ids=list(range(B)))
    _CACHE["last_res"] = res
    out = np.stack(
        [_decode_y(res.results[b]["yz"]).reshape(O, H, W_) for b in range(B)]
    )
    return out


# revision 7
# speedup vs baseline: 1.0960x; 1.0960x over previous
"""Deformable (region-aware) matmul for Trainium2, data-parallel over batch.

out[b,o,h,w] = sum_r sum_c mat0[b,c,h,w] * mat1[o,c,r] * Alpha[r] * mask[r,h,w]

Shapes: B=8, C=256, H=W=64, O=256, R=8.  One batch per NeuronCore (8 cores).

Fold (region, channel-half) into a 16-tile contraction axis: k-tile
t = 2*r + half.  Activation tile X_t[k, p] = mat0[half*128+k, p] * mask[r, p]
(Vector engine, bf16 2x, broadcast APs against a host-prebroadcast mask).
Weight tile W_t[k, o] = mat1[o, half*128+k, r]*Alpha[r] (lhsT layout).  The
Tensor engine chains out[o, p] = sum_t W_t.T @ X_t in PSUM at the bf16
roofline (213 ns / 512-col matmul, ~55 us/core total).

The schedule is built around the measured fixed costs (6.5 us NEFF preamble,
1.6 us DMA first-byte, ~2 us completion receipt): the pixel axis is cut into
slabs (256,256,512*6,256,256); slab0's whole working set rides in two
fat-descriptor "prologue bundles" on the sync ring (weights for the first
output half + x + mask interleaved) so the MM stream starts ~3.5 us after
the preamble; steady slabs are single [x|mask] bundle DMAs on the scalar
ring; weights are split by output half so only 0.5 MB gates the first chain.
PE warmup matmuls cover the prologue (HAM at 2.4 GHz when the stream
starts); output is stored bf16 and the last slab is small so the final DMA
drains quickly.
"""

import numpy as np
import ml_dtypes

B, C, H, W_ = 8, 256, 64, 64
O, R = 256, 8
P = H * W_            # 4096 pixels
KT = 2 * R            # 16 k-tiles of 128
SL = [256, 256, 512, 512, 512, 512, 512, 512, 256, 256]   # slab pixel counts
assert sum(SL) == P
SOFF = [0] * len(SL)
for _i in range(1, len(SL)):
    SOFF[_i] = SOFF[_i - 1] + SL[_i - 1]
NWARM = 7             # PE warmup matmuls (cover the ~3.3 us data lead-in)
MAXPX = max(SL)
PX0 = SL[0]
NS = len(SL)

_CACHE = {}


def _build():
    import concourse.bacc as bacc
    import concourse.tile as tile
    import concourse.mybir as mybir

    bf16 = mybir.dt.bfloat16
    f32 = mybir.dt.float32

    nc = bacc.Bacc(
        "TRN2",
        target_bir_lowering=False,
        debug=False,
        enable_asserts=False,
        num_devices=8,
    )
    # Host-prepped per-core inputs (see _prep_inputs):
    #  pba[k, :]  = [ w_m0 t0-7 (8*128) | x_s0 (2*PX0) | mb_s0 r0-3 (4*PX0) ]
    #  pbb[k, :]  = [ w_m0 t8-15 (8*128) | mb_s0 r4-7 (4*PX0) ]
    #  wm[k, m*KT*128 + t*128 + o'] = mat1[m*128+o', c(t,k), r(t)] * Alpha
    #  bz[k, :]   = per-slab s>=1: [ x (2*px) | mb (8*px) ] concatenated
    #  yz[k, 2*soff + m*px + i] = out[m*128+k, soff+i]  (bf16)
    pba_d = nc.dram_tensor("pba", [128, 8 * 128 + 6 * PX0], bf16, kind="ExternalInput")
    pbb_d = nc.dram_tensor("pbb", [128, 8 * 128 + 4 * PX0], bf16, kind="ExternalInput")
    wm_d = nc.dram_tensor("wm", [128, 2 * KT * 128], bf16, kind="ExternalInput")
    BZTOT = sum(10 * px for px in SL[1:])
    bz_d = nc.dram_tensor("bz", [128, BZTOT], bf16, kind="ExternalInput")
    y_d = nc.dram_tensor("yz", [128, 2 * P], bf16, kind="ExternalOutput")

    boff = [0] * NS  # offsets into bz_d per slab (s>=1)
    for s in range(2, NS):
        boff[s] = boff[s - 1] + 10 * SL[s - 1]

    with tile.TileContext(nc) as tc:
        with (
            tc.tile_pool(name="const", bufs=1) as cpool,
            tc.tile_pool(name="bzp", bufs=3) as bzpool,
            tc.tile_pool(name="xtp", bufs=4) as xtpool,
            tc.tile_pool(name="psp", bufs=8, space="PSUM") as pspool,
            tc.tile_pool(name="yp", bufs=3) as ypool,
        ):
            # --- prologue DMAs on the sync ring, in consumption order
            pba = cpool.tile([128, 8 * 128 + 6 * PX0], bf16, tag="pba")
            pbb = cpool.tile([128, 8 * 128 + 4 * PX0], bf16, tag="pbb")
            wm = cpool.tile([128, 2 * KT * 128], bf16, tag="wm")
            nc.sync.dma_start(out=pba[:], in_=pba_d[:])
            nc.sync.dma_start(out=pbb[:], in_=pbb_d[:])
            nc.sync.dma_start(out=wm[:, KT * 128 :], in_=wm_d[:, KT * 128 :])  # m1
            # m0 weights lead the scalar ring: they gate every chain from s1
            # on, and must not queue behind (or share bandwidth with) the
            # bundle prefetch stream.
            nc.scalar.dma_start(out=wm[:, : KT * 128], in_=wm_d[:, : KT * 128])

            # --- steady-slab bundles on the scalar ring
            bz_sb = [None] * NS
            for s in range(1, NS):
                px = SL[s]
                t_ = bzpool.tile([128, 10 * MAXPX], bf16, tag="bz")
                nc.scalar.dma_start(
                    out=t_[:, : 10 * px], in_=bz_d[:, boff[s] : boff[s] + 10 * px]
                )
                bz_sb[s] = t_

            # --- PE warmup (vector memset of one combined tile keeps it cheap)
            warm = cpool.tile([128, 640], bf16, tag="warm")
            nc.vector.memset(warm[:], 0.0)
            warm_ps = pspool.tile([128, 512], f32, tag="ps")
            for _ in range(NWARM):
                nc.tensor.matmul(
                    warm_ps[:], warm[:, :128], warm[:, 128:640], start=True, stop=True
                )

            XW = 8 * 128  # weight prefix length in pba/pbb

            def lhsT(s, m, t):
                if s == 0 and m == 0:
                    src = pba if t < 8 else pbb
                    return src[:, (t % 8) * 128 : (t % 8 + 1) * 128]
                return wm[:, m * KT * 128 + t * 128 : m * KT * 128 + (t + 1) * 128]

            for s, px in enumerate(SL):
                if s == 0:
                    x_ap = pba[:, XW : XW + 2 * px]
                    mb_lo = pba[:, XW + 2 * px : XW + 6 * px]
                    mb_hi = pbb[:, XW : XW + 4 * px]
                    gsrc = [(0, 2, mb_lo), (2, 4, mb_lo), (4, 6, mb_hi), (6, 8, mb_hi)]
                else:
                    bzt = bz_sb[s]
                    x_ap = bzt[:, : 2 * px]
                    mb = bzt[:, 2 * px : 10 * px]
                    if s <= 2:
                        gsrc = [(0, 2, mb), (2, 4, mb), (4, 6, mb), (6, 8, mb)]
                    else:
                        gsrc = [(0, 4, mb), (4, 8, mb)]
                xt = xtpool.tile([128, KT * MAXPX], bf16, tag="xt")
                for lo, hi, mbsrc in gsrc:
                    nr = hi - lo
                    # mbsrc holds rows [blo..] where blo = 0 for pba/bz, 4 for pbb
                    blo = 4 if (s == 0 and mbsrc is mb_hi) else 0
                    out_ap = (
                        xt[:, 2 * lo * px : 2 * hi * px]
                        .rearrange("q (r h f) -> q r h f", r=nr, h=2)
                    )
                    in0 = (
                        x_ap.rearrange("q (h f) -> q h f", h=2)
                        .unsqueeze(1)
                        .broadcast_to([128, nr, 2, px])
                    )
                    in1 = (
                        mbsrc[:, (lo - blo) * px : (hi - blo) * px]
                        .rearrange("q (r f) -> q r f", r=nr)
                        .unsqueeze(2)
                        .broadcast_to([128, nr, 2, px])
                    )
                    nc.vector.tensor_mul(out_ap, in0, in1)
                y_sb = ypool.tile([128, 2 * MAXPX], bf16, tag="y")
                yo = 2 * SOFF[s]
                for m in range(2):
                    ps = pspool.tile([128, 512], f32, tag="ps")
                    for t in range(KT):
                        nc.tensor.matmul(
                            ps[:, :px],
                            lhsT(s, m, t),
                            xt[:, t * px : (t + 1) * px],
                            start=(t == 0),
                            stop=(t == KT - 1),
                        )
                    nc.scalar.copy(y_sb[:, m * px : (m + 1) * px], ps[:, :px])
                    if s == NS - 1:  # split the final writes so the last is tiny
                        nc.sync.dma_start(
                            out=y_d[:, yo + m * px : yo + (m + 1) * px],
                            in_=y_sb[:, m * px : (m + 1) * px],
                        )
                if s != NS - 1:
                    nc.sync.dma_start(
                        out=y_d[:, yo : yo + 2 * px], in_=y_sb[:, : 2 * px]
                    )

    nc.compile()
    return nc


def _prep_inputs(mat0, mat1, mask, Alpha, use_alpha):
    bf = ml_dtypes.bfloat16
    m1 = mat1 * np.asarray(Alpha)[None, None, :] if int(use_alpha) else mat1
    # w[k, t, o] with t = r*2 + half, c = half*128 + k
    w = np.transpose(m1.reshape(O, 2, 128, R), (2, 3, 1, 0))  # [k, r, half, o]
    w = w.reshape(128, KT, O)
    # wm[k, m, t, o'] -> [128, 2*KT*128]
    wm = np.transpose(w.reshape(128, KT, 2, 128), (0, 2, 1, 3))
    wm_h = np.ascontiguousarray(wm.reshape(128, 2 * KT * 128)).astype(bf)

    mask2 = np.asarray(mask, dtype=np.float32).reshape(R, P)
    xr = np.asarray(mat0, dtype=np.float32).reshape(B, 2, 128, P)

    def xblk(b, s):  # [128, 2*px] = x halves for slab s
        p0, px = SOFF[s], SL[s]
        return (
            np.transpose(xr[b, :, :, p0 : p0 + px], (1, 0, 2))
            .reshape(128, 2 * px)
            .astype(bf)
        )

    def mblk(s, lo, hi):  # [128, (hi-lo)*px] mask rows broadcast
        p0, px = SOFF[s], SL[s]
        return (
            np.broadcast_to(mask2[None, lo:hi, p0 : p0 + px], (128, hi - lo, px))
            .reshape(128, (hi - lo) * px)
            .astype(bf)
        )

    wq = wm_h.reshape(128, 2, KT, 128)
    pba = np.empty((B, 128, 8 * 128 + 6 * SL[0]), dtype=bf)
    pbb = np.empty((128, 8 * 128 + 4 * SL[0]), dtype=bf)
    pbb[:, : 8 * 128] = wq[:, 0, 8:, :].reshape(128, 8 * 128)
    pbb[:, 8 * 128 :] = mblk(0, 4, 8)
    mb03 = mblk(0, 0, 4)
    for b in range(B):
        pba[b, :, : 8 * 128] = wq[:, 0, :8, :].reshape(128, 8 * 128)
        pba[b, :, 8 * 128 : 8 * 128 + 2 * SL[0]] = xblk(b, 0)
        pba[b, :, 8 * 128 + 2 * SL[0] :] = mb03

    BZTOT = sum(10 * px for px in SL[1:])
    bz = np.empty((B, 128, BZTOT), dtype=bf)
    off = 0
    for s in range(1, NS):
        px = SL[s]
        mb = mblk(s, 0, 8)
        for b in range(B):
            bz[b, :, off : off + 2 * px] = xblk(b, s)
            bz[b, :, off + 2 * px : off + 10 * px] = mb
        off += 10 * px
    return pba, pbb, wm_h, bz


def _decode_y(yz):
    # yz [128, 2*P] bf16 slab-major -> out [O, P] float32
    out = np.empty((O, P), dtype=np.float32)
    y = np.asarray(yz)
    for s, px in enumerate(SL):
        p0 = SOFF[s]
        blk = y[:, 2 * p0 : 2 * (p0 + px)].reshape(128, 2, px).astype(np.float32)
        out[:128, p0 : p0 + px] = blk[:, 0, :]
        out[128:, p0 : p0 + px] = blk[:, 1, :]
    return out


def _make_in_maps(mat0, mat1, mask, Alpha, use_alpha):
    pba, pbb, wm_h, bz = _prep_inputs(mat0, mat1, mask, Alpha, use_alpha)
    return [
        {"pba": pba[b], "pbb": pbb, "wm": wm_h, "bz": bz[b]} for b in range(B)
    ]


def kernel(mat0, mat1, mask, Alpha, use_alpha, beta):
    from concourse import bass_utils

    mat0 = np.asarray(mat0, dtype=np.float32)
    mat1 = np.asarray(mat1, dtype=np.float32)
    mask = np.asarray(mask, dtype=np.float32)
    Alpha = np.asarray(Alpha, dtype=np.float32)

    if "nc" not in _CACHE:
        _CACHE["nc"] = _build()
    nc = _CACHE["nc"]

    in_maps = _make_in_maps(mat0, mat1, mask, Alpha, use_alpha)
    res = bass_utils.run_bass_kernel_spmd(nc, in_maps, core_ids=list(range(B)))
    _CACHE["last_res"] = res
    out = np.stack(
        [_decode_y(res.results[b]["yz"]).reshape(O, H, W_) for b in range(B)]
    )
    return out


# revision 10
# speedup vs baseline: 1.1207x; 1.0226x over previous
"""Deformable (region-aware) matmul for Trainium2, data-parallel over batch.

out[b,o,h,w] = sum_r sum_c mat0[b,c,h,w] * mat1[o,c,r] * Alpha[r] * mask[r,h,w]

Shapes: B=8, C=256, H=W=64, O=256, R=8.  One batch per NeuronCore (8 cores).

Fold (region, channel-half) into a 16-tile contraction axis: k-tile
t = 2*r + half.  Activation tile X_t[k, p] = mat0[half*128+k, p] * mask[r, p]
(Vector engine, bf16 2x mode, broadcast APs against a host-prebroadcast
mask).  Weight tile W_t[k, o] = mat1[o, half*128+k, r]*Alpha[r] (lhsT
layout).  The Tensor engine chains out[o, p] = sum_t W_t.T @ X_t in PSUM at
the bf16 roofline (213 ns / 512-col matmul, ~55 us/core of pure streaming).

Schedule notes (from trace iterations): the front of the kernel is
aggregate-HBM-bound (~360 GB/s across both HWDGE rings), so the first-needed
bytes are strictly priority-ordered and split across the two rings so that
adjacent-priority pieces land together: sync carries w(m0,t0-7), w(m0,t8-15)
then slab1's [x|mask] bundle; scalar carries slab0's bundle, w(m1), then
slab2/3 bundles.  Remaining slab bundles are issued from inside earlier slab
bodies so their tile-pool-slot waits are already resolved when the ACT
sequencer reaches them (v4 lesson: queued-up bundle issues otherwise block
every PSUM evacuation behind them in ACT's strict FIFO).  The first slab is
128 px so the matmul stream starts ~1.5 us after the first bytes arrive; PE
warmup matmuls cover the lead-in so the HAM clock gate is released when the
real stream starts.  Output is stored bf16 and the final slab is small so
the last DMA drains quickly.
"""

import numpy as np
import ml_dtypes

B, C, H, W_ = 8, 256, 64, 64
O, R = 256, 8
P = H * W_            # 4096 pixels
KT = 2 * R            # 16 k-tiles of 128
SL = [128, 256, 512, 512, 512, 512, 512, 512, 384, 256]   # slab pixel counts
assert sum(SL) == P
SOFF = [0] * len(SL)
for _i in range(1, len(SL)):
    SOFF[_i] = SOFF[_i - 1] + SL[_i - 1]
NWARM = 6             # PE warmup matmuls (cover the ~2.8 us data lead-in)
MAXPX = max(SL)
NS = len(SL)

_CACHE = {}


def _build():
    import concourse.bacc as bacc
    import concourse.tile as tile
    import concourse.mybir as mybir

    bf16 = mybir.dt.bfloat16
    f32 = mybir.dt.float32

    nc = bacc.Bacc(
        "TRN2",
        target_bir_lowering=False,
        debug=False,
        enable_asserts=False,
        num_devices=8,
    )
    # Host-prepped per-core inputs (see _prep_inputs):
    #  wm[k, m*KT*128 + t*128 + o'] = mat1[m*128+o', c(t,k), r(t)] * Alpha
    #  bz[k, :] = per-slab [ x (2*px) | mb (8*px) ] concatenated over slabs
    #  yz[k, 2*soff + m*px + i] = out[m*128+k, soff+i]  (bf16)
    wm_d = nc.dram_tensor("wm", [128, 2 * KT * 128], bf16, kind="ExternalInput")
    BZTOT = sum(10 * px for px in SL)
    bz_d = nc.dram_tensor("bz", [128, BZTOT], bf16, kind="ExternalInput")
    y_d = nc.dram_tensor("yz", [128, 2 * P], bf16, kind="ExternalOutput")

    boff = [0] * NS
    for s in range(1, NS):
        boff[s] = boff[s - 1] + 10 * SL[s - 1]

    with tile.TileContext(nc) as tc:
        with (
            tc.tile_pool(name="const", bufs=1) as cpool,
            tc.tile_pool(name="bzp", bufs=5) as bzpool,
            tc.tile_pool(name="xtp", bufs=4) as xtpool,
            tc.tile_pool(name="psp", bufs=8, space="PSUM") as pspool,
            tc.tile_pool(name="yp", bufs=3) as ypool,
        ):
            wm = cpool.tile([128, 2 * KT * 128], bf16, tag="wm")
            HKT = KT // 2 * 128

            bz_sb = [None] * NS

            def issue_bz(s, engine):
                px = SL[s]
                t_ = bzpool.tile([128, 10 * MAXPX], bf16, tag="bz")
                engine.dma_start(
                    out=t_[:, : 10 * px], in_=bz_d[:, boff[s] : boff[s] + 10 * px]
                )
                bz_sb[s] = t_

            # Priority-ordered front, alternating across the two HWDGE rings:
            #  sync:   w(m0,t0-7) | w(m0,t8-15) | bz1          ... then y's
            #  scalar: bz0        | w(m1)       | bz2 | bz3    ... then evacs
            nc.sync.dma_start(out=wm[:, :HKT], in_=wm_d[:, :HKT])
            issue_bz(0, nc.scalar)
            nc.sync.dma_start(out=wm[:, HKT : 2 * HKT], in_=wm_d[:, HKT : 2 * HKT])
            nc.scalar.dma_start(out=wm[:, 2 * HKT :], in_=wm_d[:, 2 * HKT :])
            issue_bz(1, nc.sync)
            issue_bz(2, nc.scalar)
            issue_bz(3, nc.scalar)

            # PE warmup on a zeroed const tile
            warm = cpool.tile([128, 640], bf16, tag="warm")
            nc.vector.memset(warm[:], 0.0)
            warm_ps = pspool.tile([128, 512], f32, tag="ps")
            for _ in range(NWARM):
                nc.tensor.matmul(
                    warm_ps[:], warm[:, :128], warm[:, 128:640], start=True, stop=True
                )

            for s, px in enumerate(SL):
                bzt = bz_sb[s]
                x_ap = bzt[:, : 2 * px]
                mb = bzt[:, 2 * px : 10 * px]
                gsrc = [(0, 4), (4, 8)] if s >= 3 else [(0, 2), (2, 4), (4, 6), (6, 8)]
                xt = xtpool.tile([128, KT * MAXPX], bf16, tag="xt")
                for lo, hi in gsrc:
                    nr = hi - lo
                    out_ap = (
                        xt[:, 2 * lo * px : 2 * hi * px]
                        .rearrange("q (r h f) -> q r h f", r=nr, h=2)
                    )
                    in0 = (
                        x_ap.rearrange("q (h f) -> q h f", h=2)
                        .unsqueeze(1)
                        .broadcast_to([128, nr, 2, px])
                    )
                    in1 = (
                        mb[:, lo * px : hi * px]
                        .rearrange("q (r f) -> q r f", r=nr)
                        .unsqueeze(2)
                        .broadcast_to([128, nr, 2, px])
                    )
                    nc.vector.tensor_mul(out_ap, in0, in1)
                # prefetch a later slab's bundle; its pool-slot wait is
                # already resolved (freed by slab s-1's muls), so it never
                # blocks the evacs behind it on the ACT sequencer
                if s <= NS - 5:
                    issue_bz(s + 4, nc.scalar)
                y_sb = ypool.tile([128, 2 * MAXPX], bf16, tag="y")
                yo = 2 * SOFF[s]
                for m in range(2):
                    ps = pspool.tile([128, 512], f32, tag="ps")
                    for t in range(KT):
                        nc.tensor.matmul(
                            ps[:, :px],
                            wm[:, m * KT * 128 + t * 128 : m * KT * 128 + (t + 1) * 128],
                            xt[:, t * px : (t + 1) * px],
                            start=(t == 0),
                            stop=(t == KT - 1),
                        )
                    nc.scalar.copy(y_sb[:, m * px : (m + 1) * px], ps[:, :px])
                    if s == NS - 1:  # split the final writes so the last is tiny
                        nc.sync.dma_start(
                            out=y_d[:, yo + m * px : yo + (m + 1) * px],
                            in_=y_sb[:, m * px : (m + 1) * px],
                        )
                if s != NS - 1:
                    nc.sync.dma_start(
                        out=y_d[:, yo : yo + 2 * px], in_=y_sb[:, : 2 * px]
                    )

    nc.compile()
    return nc


def _prep_inputs(mat0, mat1, mask, Alpha, use_alpha):
    bf = ml_dtypes.bfloat16
    m1 = mat1 * np.asarray(Alpha)[None, None, :] if int(use_alpha) else mat1
    # w[k, t, o] with t = r*2 + half, c = half*128 + k
    w = np.transpose(m1.reshape(O, 2, 128, R), (2, 3, 1, 0))  # [k, r, half, o]
    w = w.reshape(128, KT, O)
    # wm[k, m, t, o'] -> [128, 2*KT*128]
    wm = np.transpose(w.reshape(128, KT, 2, 128), (0, 2, 1, 3))
    wm_h = np.ascontiguousarray(wm.reshape(128, 2 * KT * 128)).astype(bf)

    mask2 = np.asarray(mask, dtype=np.float32).reshape(R, P)
    xr = np.asarray(mat0, dtype=np.float32).reshape(B, 2, 128, P)

    BZTOT = sum(10 * px for px in SL)
    bz = np.empty((B, 128, BZTOT), dtype=bf)
    off = 0
    for s, px in enumerate(SL):
        p0 = SOFF[s]
        xblk = np.transpose(xr[:, :, :, p0 : p0 + px], (0, 2, 1, 3)).reshape(
            B, 128, 2 * px
        )
        mb = np.broadcast_to(mask2[None, :, p0 : p0 + px], (128, R, px)).reshape(
            128, R * px
        ).astype(bf)
        for b in range(B):
            bz[b, :, off : off + 2 * px] = xblk[b].astype(bf)
            bz[b, :, off + 2 * px : off + 10 * px] = mb
        off += 10 * px
    return wm_h, bz


def _decode_y(yz):
    # yz [128, 2*P] bf16 slab-major -> out [O, P] float32
    out = np.empty((O, P), dtype=np.float32)
    y = np.asarray(yz)
    for s, px in enumerate(SL):
        p0 = SOFF[s]
        blk = y[:, 2 * p0 : 2 * (p0 + px)].reshape(128, 2, px).astype(np.float32)
        out[:128, p0 : p0 + px] = blk[:, 0, :]
        out[128:, p0 : p0 + px] = blk[:, 1, :]
    return out


def _make_in_maps(mat0, mat1, mask, Alpha, use_alpha):
    wm_h, bz = _prep_inputs(mat0, mat1, mask, Alpha, use_alpha)
    return [{"wm": wm_h, "bz": bz[b]} for b in range(B)]


def kernel(mat0, mat1, mask, Alpha, use_alpha, beta):
    from concourse import bass_utils

    mat0 = np.asarray(mat0, dtype=np.float32)
    mat1 = np.asarray(mat1, dtype=np.float32)
    mask = np.asarray(mask, dtype=np.float32)
    Alpha = np.asarray(Alpha, dtype=np.float32)

    if "nc" not in _CACHE:
        _CACHE["nc"] = _build()
    nc = _CACHE["nc"]

    in_maps = _make_in_maps(mat0, mat1, mask, Alpha, use_alpha)
    res = bass_utils.run_bass_kernel_spmd(nc, in_maps, core_ids=list(range(B)))
    _CACHE["last_res"] = res
    out = np.stack(
        [_decode_y(res.results[b]["yz"]).reshape(O, H, W_) for b in range(B)]
    )
    return out


# revision 11
# speedup vs baseline: 1.1256x; 1.0043x over previous
"""Deformable (region-aware) matmul for Trainium2, data-parallel over batch.

out[b,o,h,w] = sum_r sum_c mat0[b,c,h,w] * mat1[o,c,r] * Alpha[r] * mask[r,h,w]

Shapes: B=8, C=256, H=W=64, O=256, R=8.  One batch per NeuronCore (8 cores).

Fold (region, channel-half) into a 16-tile contraction axis: k-tile
t = 2*r + half.  Activation tile X_t[k, p] = mat0[half*128+k, p] * mask[r, p]
(Vector engine, bf16 2x mode, broadcast APs against a host-prebroadcast
mask).  Weight tile W_t[k, o] = mat1[o, half*128+k, r]*Alpha[r] (lhsT
layout).  The Tensor engine chains out[o, p] = sum_t W_t.T @ X_t in PSUM at
the bf16 roofline (213 ns / 512-col matmul, ~55 us/core of pure streaming).

Schedule notes (from five trace iterations): the kernel front is limited by
HWDGE ramp-up + aggregate HBM bandwidth (~360 GB/s over both rings), and
many small queued DMAs ramp far slower than one large one.  So the entire
critical front rides in three fat host-packed DMAs, in strict need-order
across the two rings:
  F1 (sync)   = [ slab0 x|mask bundle | w(m0, t0-7) ]   -> first chain fed
  F2 (scalar) = [ w(m0, t8-15) | w(m1, t0-7) ]
  F3 (sync)   = [ w(m1, t8-15) | slab1 bundle ]
Chains read their lhsT weights directly out of F1/F2/F3.  Remaining slabs
are single [x|mask] bundle DMAs issued from inside earlier slab bodies so
their tile-pool-slot waits are pre-resolved and never block the PSUM
evacuations behind them in ACT's strict FIFO (v4 lesson).  Slabs are
ordered small-N first: N=128/256 chains are LDWEIGHTS-bound (~107 ns/MM at
either clock), so they absorb the cold-clock window while the HAM gate
warms; PE warmup matmuls cover the lead-in so the stream never idles.
Output is stored bf16; the final slab is small so the last DMA drains fast.
"""

import numpy as np
import ml_dtypes

B, C, H, W_ = 8, 256, 64, 64
O, R = 256, 8
P = H * W_            # 4096 pixels
KT = 2 * R            # 16 k-tiles of 128
SL = [128, 256, 512, 512, 512, 512, 512, 512, 384, 256]   # slab pixel counts
assert sum(SL) == P
SOFF = [0] * len(SL)
for _i in range(1, len(SL)):
    SOFF[_i] = SOFF[_i - 1] + SL[_i - 1]
NWARM = 7             # PE warmup matmuls bridge body-start to first data
MAXPX = max(SL)
NS = len(SL)
HW8 = 8 * 128         # one m-half x t-half of weights, per partition elems

_CACHE = {}


def _build():
    import concourse.bacc as bacc
    import concourse.tile as tile
    import concourse.mybir as mybir

    bf16 = mybir.dt.bfloat16
    f32 = mybir.dt.float32

    nc = bacc.Bacc(
        "TRN2",
        target_bir_lowering=False,
        debug=False,
        enable_asserts=False,
        num_devices=8,
    )
    # Host-packed fronts (see _prep_inputs):
    #  f1[k] = [ x_s0 (2*128) | mb_s0 (8*128) | w(m0,t0-7) (8*128) ]
    #  f2[k] = [ w(m0,t8-15) (8*128) | w(m1,t0-7) (8*128) ]
    #  f3[k] = [ w(m1,t8-15) (8*128) | x_s1 (2*256) | mb_s1 (8*256) ]
    #  bz[k] = per-slab s>=2: [ x (2*px) | mb (8*px) ] concatenated
    #  yz[k, 2*soff + m*px + i] = out[m*128+k, soff+i]  (bf16)
    f1_d = nc.dram_tensor("f1", [128, 10 * SL[0] + HW8], bf16, kind="ExternalInput")
    f2_d = nc.dram_tensor("f2", [128, 2 * HW8], bf16, kind="ExternalInput")
    f3_d = nc.dram_tensor("f3", [128, HW8 + 10 * SL[1]], bf16, kind="ExternalInput")
    BZTOT = sum(10 * px for px in SL[2:])
    bz_d = nc.dram_tensor("bz", [128, BZTOT], bf16, kind="ExternalInput")
    y_d = nc.dram_tensor("yz", [128, 2 * P], bf16, kind="ExternalOutput")

    boff = [0] * NS
    for s in range(3, NS):
        boff[s] = boff[s - 1] + 10 * SL[s - 1]

    with tile.TileContext(nc) as tc:
        with (
            tc.tile_pool(name="const", bufs=1) as cpool,
            tc.tile_pool(name="bzp", bufs=5) as bzpool,
            tc.tile_pool(name="xtp", bufs=4) as xtpool,
            tc.tile_pool(name="psp", bufs=8, space="PSUM") as pspool,
            tc.tile_pool(name="yp", bufs=3) as ypool,
        ):
            f1 = cpool.tile([128, 10 * SL[0] + HW8], bf16, tag="f1")
            f2 = cpool.tile([128, 2 * HW8], bf16, tag="f2")
            f3 = cpool.tile([128, HW8 + 10 * SL[1]], bf16, tag="f3")

            bz_sb = [None] * NS

            def issue_bz(s, engine):
                px = SL[s]
                t_ = bzpool.tile([128, 10 * MAXPX], bf16, tag="bz")
                engine.dma_start(
                    out=t_[:, : 10 * px], in_=bz_d[:, boff[s] : boff[s] + 10 * px]
                )
                bz_sb[s] = t_

            nc.sync.dma_start(out=f1[:], in_=f1_d[:])
            nc.scalar.dma_start(out=f2[:], in_=f2_d[:])
            nc.sync.dma_start(out=f3[:], in_=f3_d[:])
            issue_bz(2, nc.scalar)
            issue_bz(3, nc.sync)
            issue_bz(4, nc.scalar)

            # PE warmup on a zeroed const tile
            warm = cpool.tile([128, 640], bf16, tag="warm")
            nc.vector.memset(warm[:], 0.0)
            warm_ps = pspool.tile([128, 512], f32, tag="ps")
            for _ in range(NWARM):
                nc.tensor.matmul(
                    warm_ps[:], warm[:, :128], warm[:, 128:640], start=True, stop=True
                )

            def lhsT(m, t):
                if m == 0:
                    if t < 8:
                        return f1[:, 10 * SL[0] + t * 128 : 10 * SL[0] + (t + 1) * 128]
                    return f2[:, (t - 8) * 128 : (t - 7) * 128]
                if t < 8:
                    return f2[:, HW8 + t * 128 : HW8 + (t + 1) * 128]
                return f3[:, (t - 8) * 128 : (t - 7) * 128]

            for s, px in enumerate(SL):
                if s == 0:
                    x_ap = f1[:, : 2 * px]
                    mb = f1[:, 2 * px : 10 * px]
                elif s == 1:
                    x_ap = f3[:, HW8 : HW8 + 2 * px]
                    mb = f3[:, HW8 + 2 * px : HW8 + 10 * px]
                else:
                    bzt = bz_sb[s]
                    x_ap = bzt[:, : 2 * px]
                    mb = bzt[:, 2 * px : 10 * px]
                gsrc = [(0, 4), (4, 8)] if s >= 3 else [(0, 2), (2, 4), (4, 6), (6, 8)]
                xt = xtpool.tile([128, KT * MAXPX], bf16, tag="xt")
                for lo, hi in gsrc:
                    nr = hi - lo
                    out_ap = (
                        xt[:, 2 * lo * px : 2 * hi * px]
                        .rearrange("q (r h f) -> q r h f", r=nr, h=2)
                    )
                    in0 = (
                        x_ap.rearrange("q (h f) -> q h f", h=2)
                        .unsqueeze(1)
                        .broadcast_to([128, nr, 2, px])
                    )
                    in1 = (
                        mb[:, lo * px : hi * px]
                        .rearrange("q (r f) -> q r f", r=nr)
                        .unsqueeze(2)
                        .broadcast_to([128, nr, 2, px])
                    )
                    nc.vector.tensor_mul(out_ap, in0, in1)
                # prefetch a later slab's bundle; pool-slot wait pre-resolved
                if 1 <= s <= NS - 5:
                    issue_bz(s + 4, nc.scalar)
                y_sb = ypool.tile([128, 2 * MAXPX], bf16, tag="y")
                yo = 2 * SOFF[s]
                for m in range(2):
                    ps = pspool.tile([128, 512], f32, tag="ps")
                    for t in range(KT):
                        nc.tensor.matmul(
                            ps[:, :px],
                            lhsT(m, t),
                            xt[:, t * px : (t + 1) * px],
                            start=(t == 0),
                            stop=(t == KT - 1),
                        )
                    nc.scalar.copy(y_sb[:, m * px : (m + 1) * px], ps[:, :px])
                    if s == NS - 1:  # split the final writes so the last is tiny
                        nc.sync.dma_start(
                            out=y_d[:, yo + m * px : yo + (m + 1) * px],
                            in_=y_sb[:, m * px : (m + 1) * px],
                        )
                if s != NS - 1:
                    nc.sync.dma_start(
                        out=y_d[:, yo : yo + 2 * px], in_=y_sb[:, : 2 * px]
                    )

    nc.compile()
    return nc


def _prep_inputs(mat0, mat1, mask, Alpha, use_alpha):
    bf = ml_dtypes.bfloat16
    m1 = mat1 * np.asarray(Alpha)[None, None, :] if int(use_alpha) else mat1
    # w[k, t, o] with t = r*2 + half, c = half*128 + k
    w = np.transpose(m1.reshape(O, 2, 128, R), (2, 3, 1, 0))  # [k, r, half, o]
    w = w.reshape(128, KT, O)
    wq = np.transpose(w.reshape(128, KT, 2, 128), (0, 2, 1, 3))  # [k, m, t, o']
    wq = wq.astype(bf)

    mask2 = np.asarray(mask, dtype=np.float32).reshape(R, P)
    xr = np.asarray(mat0, dtype=np.float32).reshape(B, 2, 128, P)

    def xblk(s):  # [B, 128, 2*px]
        p0, px = SOFF[s], SL[s]
        return np.transpose(xr[:, :, :, p0 : p0 + px], (0, 2, 1, 3)).reshape(
            B, 128, 2 * px
        ).astype(bf)

    def mblk(s):  # [128, 8*px]
        p0, px = SOFF[s], SL[s]
        return np.broadcast_to(
            mask2[None, :, p0 : p0 + px], (128, R, px)
        ).reshape(128, R * px).astype(bf)

    f1 = np.empty((B, 128, 10 * SL[0] + HW8), dtype=bf)
    f1[:, :, : 2 * SL[0]] = xblk(0)
    f1[:, :, 2 * SL[0] : 10 * SL[0]] = mblk(0)[None]
    f1[:, :, 10 * SL[0] :] = wq[:, 0, :8, :].reshape(128, HW8)[None]

    f2 = np.empty((128, 2 * HW8), dtype=bf)
    f2[:, :HW8] = wq[:, 0, 8:, :].reshape(128, HW8)
    f2[:, HW8:] = wq[:, 1, :8, :].reshape(128, HW8)

    f3 = np.empty((B, 128, HW8 + 10 * SL[1]), dtype=bf)
    f3[:, :, :HW8] = wq[:, 1, 8:, :].reshape(128, HW8)[None]
    f3[:, :, HW8 : HW8 + 2 * SL[1]] = xblk(1)
    f3[:, :, HW8 + 2 * SL[1] :] = mblk(1)[None]

    BZTOT = sum(10 * px for px in SL[2:])
    bz = np.empty((B, 128, BZTOT), dtype=bf)
    off = 0
    for s in range(2, NS):
        px = SL[s]
        xb = xblk(s)
        mb = mblk(s)
        bz[:, :, off : off + 2 * px] = xb
        bz[:, :, off + 2 * px : off + 10 * px] = mb[None]
        off += 10 * px
    return f1, f2, f3, bz


def _decode_y(yz):
    # yz [128, 2*P] bf16 slab-major -> out [O, P] float32
    out = np.empty((O, P), dtype=np.float32)
    y = np.asarray(yz)
    for s, px in enumerate(SL):
        p0 = SOFF[s]
        blk = y[:, 2 * p0 : 2 * (p0 + px)].reshape(128, 2, px).astype(np.float32)
        out[:128, p0 : p0 + px] = blk[:, 0, :]
        out[128:, p0 : p0 + px] = blk[:, 1, :]
    return out


def _make_in_maps(mat0, mat1, mask, Alpha, use_alpha):
    f1, f2, f3, bz = _prep_inputs(mat0, mat1, mask, Alpha, use_alpha)
    return [
        {"f1": f1[b], "f2": f2, "f3": f3[b], "bz": bz[b]} for b in range(B)
    ]


def kernel(mat0, mat1, mask, Alpha, use_alpha, beta):
    from concourse import bass_utils

    mat0 = np.asarray(mat0, dtype=np.float32)
    mat1 = np.asarray(mat1, dtype=np.float32)
    mask = np.asarray(mask, dtype=np.float32)
    Alpha = np.asarray(Alpha, dtype=np.float32)

    if "nc" not in _CACHE:
        _CACHE["nc"] = _build()
    nc = _CACHE["nc"]

    in_maps = _make_in_maps(mat0, mat1, mask, Alpha, use_alpha)
    res = bass_utils.run_bass_kernel_spmd(nc, in_maps, core_ids=list(range(B)))
    _CACHE["last_res"] = res
    out = np.stack(
        [_decode_y(res.results[b]["yz"]).reshape(O, H, W_) for b in range(B)]
    )
    return out


# revision 12
# speedup vs baseline: 1.1416x; 1.0143x over previous
"""Deformable (region-aware) matmul for Trainium2, data-parallel over batch.

out[b,o,h,w] = sum_r sum_c mat0[b,c,h,w] * mat1[o,c,r] * Alpha[r] * mask[r,h,w]

Shapes: B=8, C=256, H=W=64, O=256, R=8.  One batch per NeuronCore (8 cores).

Fold (region, channel-half) into a 16-tile contraction axis: k-tile
t = 2*r + half.  Activation tile X_t[k, p] = mat0[half*128+k, p] * mask[r, p]
(Vector engine, bf16 2x mode, broadcast APs against a host-prebroadcast
mask).  Weight tile W_t[k, o] = mat1[o, half*128+k, r]*Alpha[r] (lhsT
layout).  The Tensor engine chains out[o, p] = sum_t W_t.T @ X_t in PSUM at
the bf16 roofline (213 ns / 512-col matmul, ~55 us/core of pure streaming).

Schedule notes (from five trace iterations): the kernel front is limited by
HWDGE ramp-up + aggregate HBM bandwidth (~360 GB/s over both rings), and
many small queued DMAs ramp far slower than one large one.  So the entire
critical front rides in three fat host-packed DMAs, in strict need-order
across the two rings:
  F1 = [ slab0 x|mask bundle | w(m0, t0-7) ]   -> first chain fed
  F2 = [ w(m0, t8-15) | w(m1, t0-7) ]
  F3 = [ w(m1, t8-15) | slab1 bundle ]
ALL DMAs ride one HWDGE ring (sync): a single DMA already fans out over all
16 SDMA engines at full HBM rate, and one FIFO queue enforces the global
priority order exactly (two concurrent rings fair-share the ~360 GB/s and
let prefetch starve the critical path - the v3/v4/v6 failure mode).
Chains read their lhsT weights directly out of F1/F2/F3.  Remaining slabs
are single [x|mask] bundle DMAs issued from inside earlier slab bodies so
their tile-pool-slot waits are pre-resolved and never block the PSUM
evacuations behind them in ACT's strict FIFO (v4 lesson).  Slabs are
ordered small-N first: N=128/256 chains are LDWEIGHTS-bound (~107 ns/MM at
either clock), so they absorb the cold-clock window while the HAM gate
warms; PE warmup matmuls cover the lead-in so the stream never idles.
Output is stored bf16; the final slab is small so the last DMA drains fast.
"""

import numpy as np
import ml_dtypes

B, C, H, W_ = 8, 256, 64, 64
O, R = 256, 8
P = H * W_            # 4096 pixels
KT = 2 * R            # 16 k-tiles of 128
SL = [128, 256, 512, 512, 512, 512, 512, 512, 384, 256]   # slab pixel counts
assert sum(SL) == P
SOFF = [0] * len(SL)
for _i in range(1, len(SL)):
    SOFF[_i] = SOFF[_i - 1] + SL[_i - 1]
NWARM = 7             # PE warmup matmuls bridge body-start to first data
MAXPX = max(SL)
NS = len(SL)
HW8 = 8 * 128         # one m-half x t-half of weights, per partition elems

_CACHE = {}


def _build():
    import concourse.bacc as bacc
    import concourse.tile as tile
    import concourse.mybir as mybir

    bf16 = mybir.dt.bfloat16
    f32 = mybir.dt.float32

    nc = bacc.Bacc(
        "TRN2",
        target_bir_lowering=False,
        debug=False,
        enable_asserts=False,
        num_devices=8,
    )
    # Host-packed fronts (see _prep_inputs):
    #  f1[k] = [ x_s0 (2*128) | mb_s0 (8*128) | w(m0,t0-7) (8*128) ]
    #  f2[k] = [ w(m0,t8-15) (8*128) | w(m1,t0-7) (8*128) ]
    #  f3[k] = [ w(m1,t8-15) (8*128) | x_s1 (2*256) | mb_s1 (8*256) ]
    #  bz[k] = per-slab s>=2: [ x (2*px) | mb (8*px) ] concatenated
    #  yz[k, 2*soff + m*px + i] = out[m*128+k, soff+i]  (bf16)
    f1_d = nc.dram_tensor("f1", [128, 10 * SL[0] + HW8], bf16, kind="ExternalInput")
    f2_d = nc.dram_tensor("f2", [128, 2 * HW8], bf16, kind="ExternalInput")
    f3_d = nc.dram_tensor("f3", [128, HW8 + 10 * SL[1]], bf16, kind="ExternalInput")
    BZTOT = sum(10 * px for px in SL[2:])
    bz_d = nc.dram_tensor("bz", [128, BZTOT], bf16, kind="ExternalInput")
    y_d = nc.dram_tensor("yz", [128, 2 * P], bf16, kind="ExternalOutput")

    boff = [0] * NS
    for s in range(3, NS):
        boff[s] = boff[s - 1] + 10 * SL[s - 1]

    with tile.TileContext(nc) as tc:
        with (
            tc.tile_pool(name="const", bufs=1) as cpool,
            tc.tile_pool(name="bzp", bufs=5) as bzpool,
            tc.tile_pool(name="xtp", bufs=4) as xtpool,
            tc.tile_pool(name="psp", bufs=8, space="PSUM") as pspool,
            tc.tile_pool(name="yp", bufs=3) as ypool,
        ):
            f1 = cpool.tile([128, 10 * SL[0] + HW8], bf16, tag="f1")
            f2 = cpool.tile([128, 2 * HW8], bf16, tag="f2")
            f3 = cpool.tile([128, HW8 + 10 * SL[1]], bf16, tag="f3")

            bz_sb = [None] * NS

            def issue_bz(s, engine):
                px = SL[s]
                t_ = bzpool.tile([128, 10 * MAXPX], bf16, tag="bz")
                engine.dma_start(
                    out=t_[:, : 10 * px], in_=bz_d[:, boff[s] : boff[s] + 10 * px]
                )
                bz_sb[s] = t_

            nc.sync.dma_start(out=f1[:], in_=f1_d[:])
            nc.sync.dma_start(out=f2[:], in_=f2_d[:])
            nc.sync.dma_start(out=f3[:], in_=f3_d[:])
            issue_bz(2, nc.sync)
            issue_bz(3, nc.sync)
            issue_bz(4, nc.sync)

            # PE warmup on a zeroed const tile
            warm = cpool.tile([128, 640], bf16, tag="warm")
            nc.vector.memset(warm[:], 0.0)
            warm_ps = pspool.tile([128, 512], f32, tag="ps")
            for _ in range(NWARM):
                nc.tensor.matmul(
                    warm_ps[:], warm[:, :128], warm[:, 128:640], start=True, stop=True
                )

            def lhsT(m, t):
                if m == 0:
                    if t < 8:
                        return f1[:, 10 * SL[0] + t * 128 : 10 * SL[0] + (t + 1) * 128]
                    return f2[:, (t - 8) * 128 : (t - 7) * 128]
                if t < 8:
                    return f2[:, HW8 + t * 128 : HW8 + (t + 1) * 128]
                return f3[:, (t - 8) * 128 : (t - 7) * 128]

            for s, px in enumerate(SL):
                if s == 0:
                    x_ap = f1[:, : 2 * px]
                    mb = f1[:, 2 * px : 10 * px]
                elif s == 1:
                    x_ap = f3[:, HW8 : HW8 + 2 * px]
                    mb = f3[:, HW8 + 2 * px : HW8 + 10 * px]
                else:
                    bzt = bz_sb[s]
                    x_ap = bzt[:, : 2 * px]
                    mb = bzt[:, 2 * px : 10 * px]
                gsrc = [(0, 4), (4, 8)] if s >= 3 else [(0, 2), (2, 4), (4, 6), (6, 8)]
                xt = xtpool.tile([128, KT * MAXPX], bf16, tag="xt")
                for lo, hi in gsrc:
                    nr = hi - lo
                    out_ap = (
                        xt[:, 2 * lo * px : 2 * hi * px]
                        .rearrange("q (r h f) -> q r h f", r=nr, h=2)
                    )
                    in0 = (
                        x_ap.rearrange("q (h f) -> q h f", h=2)
                        .unsqueeze(1)
                        .broadcast_to([128, nr, 2, px])
                    )
                    in1 = (
                        mb[:, lo * px : hi * px]
                        .rearrange("q (r f) -> q r f", r=nr)
                        .unsqueeze(2)
                        .broadcast_to([128, nr, 2, px])
                    )
                    nc.vector.tensor_mul(out_ap, in0, in1)
                # prefetch a later slab's bundle; pool-slot wait pre-resolved
                if 1 <= s <= NS - 5:
                    issue_bz(s + 4, nc.sync)
                y_sb = ypool.tile([128, 2 * MAXPX], bf16, tag="y")
                yo = 2 * SOFF[s]
                for m in range(2):
                    ps = pspool.tile([128, 512], f32, tag="ps")
                    for t in range(KT):
                        nc.tensor.matmul(
                            ps[:, :px],
                            lhsT(m, t),
                            xt[:, t * px : (t + 1) * px],
                            start=(t == 0),
                            stop=(t == KT - 1),
                        )
                    nc.scalar.copy(y_sb[:, m * px : (m + 1) * px], ps[:, :px])
                    if s == NS - 1:  # split the final writes so the last is tiny
                        nc.sync.dma_start(
                            out=y_d[:, yo + m * px : yo + (m + 1) * px],
                            in_=y_sb[:, m * px : (m + 1) * px],
                        )
                if s != NS - 1:
                    nc.sync.dma_start(
                        out=y_d[:, yo : yo + 2 * px], in_=y_sb[:, : 2 * px]
                    )

    nc.compile()
    return nc


def _prep_inputs(mat0, mat1, mask, Alpha, use_alpha):
    bf = ml_dtypes.bfloat16
    m1 = mat1 * np.asarray(Alpha)[None, None, :] if int(use_alpha) else mat1
    # w[k, t, o] with t = r*2 + half, c = half*128 + k
    w = np.transpose(m1.reshape(O, 2, 128, R), (2, 3, 1, 0))  # [k, r, half, o]
    w = w.reshape(128, KT, O)
    wq = np.transpose(w.reshape(128, KT, 2, 128), (0, 2, 1, 3))  # [k, m, t, o']
    wq = wq.astype(bf)

    mask2 = np.asarray(mask, dtype=np.float32).reshape(R, P)
    xr = np.asarray(mat0, dtype=np.float32).reshape(B, 2, 128, P)

    def xblk(s):  # [B, 128, 2*px]
        p0, px = SOFF[s], SL[s]
        return np.transpose(xr[:, :, :, p0 : p0 + px], (0, 2, 1, 3)).reshape(
            B, 128, 2 * px
        ).astype(bf)

    def mblk(s):  # [128, 8*px]
        p0, px = SOFF[s], SL[s]
        return np.broadcast_to(
            mask2[None, :, p0 : p0 + px], (128, R, px)
        ).reshape(128, R * px).astype(bf)

    f1 = np.empty((B, 128, 10 * SL[0] + HW8), dtype=bf)
    f1[:, :, : 2 * SL[0]] = xblk(0)
    f1[:, :, 2 * SL[0] : 10 * SL[0]] = mblk(0)[None]
    f1[:, :, 10 * SL[0] :] = wq[:, 0, :8, :].reshape(128, HW8)[None]

    f2 = np.empty((128, 2 * HW8), dtype=bf)
    f2[:, :HW8] = wq[:, 0, 8:, :].reshape(128, HW8)
    f2[:, HW8:] = wq[:, 1, :8, :].reshape(128, HW8)

    f3 = np.empty((B, 128, HW8 + 10 * SL[1]), dtype=bf)
    f3[:, :, :HW8] = wq[:, 1, 8:, :].reshape(128, HW8)[None]
    f3[:, :, HW8 : HW8 + 2 * SL[1]] = xblk(1)
    f3[:, :, HW8 + 2 * SL[1] :] = mblk(1)[None]

    BZTOT = sum(10 * px for px in SL[2:])
    bz = np.empty((B, 128, BZTOT), dtype=bf)
    off = 0
    for s in range(2, NS):
        px = SL[s]
        xb = xblk(s)
        mb = mblk(s)
        bz[:, :, off : off + 2 * px] = xb
        bz[:, :, off + 2 * px : off + 10 * px] = mb[None]
        off += 10 * px
    return f1, f2, f3, bz


def _decode_y(yz):
    # yz [128, 2*P] bf16 slab-major -> out [O, P] float32
    out = np.empty((O, P), dtype=np.float32)
    y = np.asarray(yz)
    for s, px in enumerate(SL):
        p0 = SOFF[s]
        blk = y[:, 2 * p0 : 2 * (p0 + px)].reshape(128, 2, px).astype(np.float32)
        out[:128, p0 : p0 + px] = blk[:, 0, :]
        out[128:, p0 : p0 + px] = blk[:, 1, :]
    return out


def _make_in_maps(mat0, mat1, mask, Alpha, use_alpha):
    f1, f2, f3, bz = _prep_inputs(mat0, mat1, mask, Alpha, use_alpha)
    return [
        {"f1": f1[b], "f2": f2, "f3": f3[b], "bz": bz[b]} for b in range(B)
    ]


def kernel(mat0, mat1, mask, Alpha, use_alpha, beta):
    from concourse import bass_utils

    mat0 = np.asarray(mat0, dtype=np.float32)
    mat1 = np.asarray(mat1, dtype=np.float32)
    mask = np.asarray(mask, dtype=np.float32)
    Alpha = np.asarray(Alpha, dtype=np.float32)

    if "nc" not in _CACHE:
        _CACHE["nc"] = _build()
    nc = _CACHE["nc"]

    in_maps = _make_in_maps(mat0, mat1, mask, Alpha, use_alpha)
    res = bass_utils.run_bass_kernel_spmd(nc, in_maps, core_ids=list(range(B)))
    _CACHE["last_res"] = res
    out = np.stack(
        [_decode_y(res.results[b]["yz"]).reshape(O, H, W_) for b in range(B)]
    )
    return out
